# revision 24
# baseline (speedup 1.0000x reference)
"""Trainium2 Bass kernel for nn_CrossAttention (b=2, s1=2048, s2=1024, H=16, hd=64).

Sharding: 8 cores = 2 batches x 4 head-groups (4 heads each).
Per-core device program (bf16 matmul datapath, fp32 PSUM accumulation):
  - PE warmup burst opens the HAM clock-gate (2.4 GHz) while the batched
    input DMAs land (the DMA rings pay ~1.5us per descriptor, so inputs
    ship as ~16 large transfers split across the sync/scalar/gpsimd rings
    instead of ~50 small ones).
  - qT proj: qT[c, s1] = q_w_g @ x_b^T            (channel-partition layout)
  - kT proj: kT[c, s2] = k_w_g @ y_b^T
  - v  proj: v[s2, c]  = y_b @ v_w_g^T            (seq-partition layout)
  - QK layernorm over head_dim (= partitions): stats via selector-matmul
    (1/hd prescaled on host so pss == mu, psq == E[x^2]);
    rstd = Exp(-0.5*Ln(var+eps)) on ACT -- no reciprocal, no sqrt table
    set; ln/exp/square/identity all live in one ACT table set with the
    softmax exp. Per-position affine (A = rstd, B' = w*B + b) broadcast to
    128 partitions via K=4/K=5 matmuls whose selectors carry the LN
    weight/bias (host-folded); the apply is two DVE passes from PSUM for
    the first head-pair, and runs from SBUF copies for the second pair so
    it can overlap attention without pinning PSUM banks.
  - RoPE on q in place: head-dim pre-permuted (evens then odds) on host;
    even<->odd swap via SBUF->SBUF DMAs (split across two rings),
    sin-multiply on GPSIMD, cos-multiply + add on DVE.
  - Attention per head-pair (c): psc_a/psc_b [128,1024] + pso_a/pso_b
    [65,1024] = exactly 8 PSUM banks; per s2-chunk the two heads' score
    matmuls alternate PE row-tiles (0,0)/(64,0) and the exp stream keeps
    ACT saturated; AV accumulation for chunk m-1 runs in PE slack; each
    pair's softmax normalize is interleaved right behind its AV.
  - softmax denominator = ones-column of v_aug (v is bias-free; the bias
    is restored at normalize time as o/den + vb); the reciprocal is
    DMA-broadcast via a DRAM bounce (stride-0 APs are DRAM-source only).
  - out-projT partial [cout, s1] = o_wt-slices @ onorm, stored fp16;
    host sums 4 partials in fp32.

The emitted BIR is post-processed to split multi-semaphore waits into
single-wait NOP chains (this walrus build allows only 1 sync-wait on
self-loading matmults and very few elsewhere).
"""

import numpy as np

B, S1, S2, CIN, H, HD = 2, 2048, 1024, 1024, 16, 64
HPC = 4                # heads per core
CPC = HPC * HD         # 256 channels per core
P = 128
KC = CIN // P          # 8 cin chunks
MC = CPC // P          # 2 channel chunks
NQ = S1 // 512         # 4 s1 slices
NK = S2 // 512         # 2 s2 slices
M2 = S2 // P           # 8 s2 chunks
SCALE = HD ** -0.5
EPS = 1e-6

_NC_CACHE = {}


def _legalize_waits(nc, mybir, limit=1):
    """Split instructions carrying >limit semaphore waits into a chain of
    single-wait NOPs on the same engine followed by the instruction."""
    n_split = 0
    for fn in nc.m.functions:
        for bb in fn.blocks:
            out = []
            for inst in bb.instructions:
                si = inst.sync_info
                waits = list(si.on_wait) if si is not None and si.on_wait else []
                if len(waits) > limit:
                    for i, w in enumerate(waits[:-limit]):
                        nop = mybir.InstNoOp(
                            name=f"{inst.name}-lw{i}", ins=[], outs=[])
                        nop.engine = inst.engine
                        nop.sync_info = mybir.SyncInfo(on_wait=[w], on_update=[])
                        try:
                            nc.register_instruction(nop, overwrite=True)
                        except Exception:
                            pass
                        out.append(nop)
                    inst.sync_info = mybir.SyncInfo(
                        on_wait=waits[-limit:], on_update=list(si.on_update))
                    n_split += 1
                out.append(inst)
            bb.instructions = out
    return n_split


def _build_nc():
    from contextlib import ExitStack

    import concourse.bass as bass
    import concourse.mybir as mybir
    import concourse.tile as tile

    f32 = mybir.dt.float32
    f16 = mybir.dt.float16
    bf16 = mybir.dt.bfloat16
    AF = mybir.ActivationFunctionType
    OP = mybir.AluOpType

    nc = bass.Bass()

    def din(name, shape, dt=bf16):
        return nc.dram_tensor(name, list(shape), dt, kind="ExternalInput")

    xT = din("xT", (CIN, S1))
    yT = din("yT", (CIN, S2))
    qwt = din("qwt", (CIN, CPC))
    kwt = din("kwt", (CIN, CPC))
    vwt = din("vwt", (CIN, CPC))
    owt = din("owt", (CPC, CIN))
    qkb = din("qkb", (P, 2, MC), f32)       # packed per-partition q/k biases
    vbhd = din("vbhd", (HD, HPC), f32)      # v bias, [head_dim, head]
    trig = din("trig", (P, 2, S1))          # packed cos/sin tables
    seld = din("sel", (CPC, HPC))           # 1/HD prescaled head selector
    qselwb = din("qselwb", (5, CPC))        # rows 0-3: w-scaled bcast sel,
    kselwb = din("kselwb", (5, CPC))        # row 4: LN bias b
    outT = nc.dram_tensor("outT", [CIN, S1], f16, kind="ExternalOutput")
    rscr = nc.dram_tensor("rscr", [P, 64], bf16, kind="Internal")

    with tile.TileContext(nc) as tc, ExitStack() as ctx:
        consts = ctx.enter_context(tc.tile_pool(name="consts", bufs=1))
        pers = ctx.enter_context(tc.tile_pool(name="pers", bufs=1))
        xs = ctx.enter_context(tc.tile_pool(name="xs", bufs=2))
        tmp = ctx.enter_context(tc.tile_pool(name="tmp", bufs=3))
        rop = ctx.enter_context(tc.tile_pool(name="rop", bufs=3))
        expp = ctx.enter_context(tc.tile_pool(name="expp", bufs=3))
        ctxA = ctx.enter_context(ExitStack())
        pp = ctxA.enter_context(tc.tile_pool(name="pp", bufs=2, space="PSUM"))
        pst = ctxA.enter_context(tc.tile_pool(name="pst", bufs=1, space="PSUM"))

        # ---- PE warmup: dummy matmuls open the HAM clock gate ----
        wdum = consts.tile([P, P], bf16)
        nc.vector.memset(wdum, 0.0)
        wps = pp.tile([P, P], f32, name="warm", tag="pp")
        for i in range(36):
            nc.tensor.matmul(wps[:], wdum[:], wdum[:], start=True, stop=True)

        # ---- constants / inputs: few, large transfers per ring ----
        qwt_sb = consts.tile([P, KC, CPC], bf16)
        yT_sb = consts.tile([P, KC, S2], bf16)
        kwt_sb = consts.tile([P, KC, CPC], bf16)
        vwt_sb = consts.tile([P, KC, CPC], bf16)
        xt0 = xs.tile([P, KC, 1024], bf16, name="xt0", tag="xs")
        xt1 = xs.tile([P, KC, 1024], bf16, name="xt1", tag="xs")
        xTr = xT.rearrange("(k p) s -> p k s", p=P)
        yTr = yT.rearrange("(k p) s -> p k s", p=P)
        # sync ring
        nc.sync.dma_start(out=qwt_sb, in_=qwt.rearrange("(k p) m -> p k m", p=P))
        nc.sync.dma_start(out=xt0[:, 0:4, :], in_=xTr[:, 0:4, 0:1024])
        nc.sync.dma_start(out=yT_sb[:, 0:4, :], in_=yTr[:, 0:4, :])
        nc.sync.dma_start(out=xt1[:, 0:4, :], in_=xTr[:, 0:4, 1024:2048])
        nc.sync.dma_start(out=vwt_sb, in_=vwt.rearrange("(k p) m -> p k m", p=P))
        # scalar ring
        nc.scalar.dma_start(out=xt0[:, 4:8, :], in_=xTr[:, 4:8, 0:1024])
        nc.scalar.dma_start(out=yT_sb[:, 4:8, :], in_=yTr[:, 4:8, :])
        nc.scalar.dma_start(out=kwt_sb, in_=kwt.rearrange("(k p) m -> p k m", p=P))
        sel_sb = consts.tile([P, MC, HPC], bf16)
        nc.scalar.dma_start(out=sel_sb, in_=seld.rearrange("(c p) h -> p c h", p=P))
        qselwb_sb = consts.tile([5, CPC], bf16)
        nc.scalar.dma_start(out=qselwb_sb, in_=qselwb[:])
        kselwb_sb = consts.tile([5, CPC], bf16)
        nc.scalar.dma_start(out=kselwb_sb, in_=kselwb[:])
        nc.scalar.dma_start(out=xt1[:, 4:8, :], in_=xTr[:, 4:8, 1024:2048])
        # gpsimd ring
        trig_sb = consts.tile([P, 2, S1], bf16)
        nc.gpsimd.dma_start(out=trig_sb, in_=trig[:])
        cosf_sb = trig_sb[:, 0, :]
        sinf_sb = trig_sb[:, 1, :]
        owt_sb = consts.tile([P, MC, CIN], bf16)
        nc.gpsimd.dma_start(out=owt_sb, in_=owt.rearrange("(c p) m -> p c m", p=P))
        qkb_sb = consts.tile([P, 2, MC], f32)
        nc.gpsimd.dma_start(out=qkb_sb, in_=qkb[:])
        qb_sb = qkb_sb[:, 0, :]
        kb_sb = qkb_sb[:, 1, :]
        vbhd_sb = consts.tile([HD, HPC], f32)
        nc.gpsimd.dma_start(out=vbhd_sb, in_=vbhd[:])
        eps4 = consts.tile([HPC, 1], f32)
        nc.vector.memset(eps4, EPS)

        # ---- persistent activations ----
        qT_sb = pers.tile([P, MC, S1], bf16)
        kT_sb = pers.tile([P, MC, S2], bf16)
        v_sb = pers.tile([P, M2, HPC, HD + 1], bf16)
        onorm = pers.tile([P, MC, S1], bf16)
        # the v ones-column has no deps; write it before anything needs it
        for m in range(M2):
            nc.vector.memset(v_sb[:, m, :, HD:HD + 1], 1.0)

        # ---- q projection (one s1-half at a time, c-major) ----
        def qproj_half(half, xt):
            for c in range(MC):
                ps = [pp.tile([P, 512], f32, name=f"psq{c}_{half}{n}", tag="pp")
                      for n in range(2)]
                for k in range(KC):
                    for n in range(2):
                        nc.tensor.matmul(
                            ps[n][:], qwt_sb[:, k, c * P:(c + 1) * P],
                            xt[:, k, n * 512:(n + 1) * 512],
                            start=(k == 0), stop=(k == KC - 1))
                for n in range(2):
                    sl = slice(half * 1024 + n * 512, half * 1024 + (n + 1) * 512)
                    nc.scalar.activation(out=qT_sb[:, c, sl], in_=ps[n][:],
                                         func=AF.Identity,
                                         bias=qb_sb[:, c:c + 1], scale=1.0)

        def kproj_n(n):
            sl = slice(n * 512, (n + 1) * 512)
            ps = [pp.tile([P, 512], f32, name=f"psk{c}_{n}", tag="pp")
                  for c in range(MC)]
            for k in range(KC):
                for c in range(MC):
                    nc.tensor.matmul(
                        ps[c][:], kwt_sb[:, k, c * P:(c + 1) * P],
                        yT_sb[:, k, sl],
                        start=(k == 0), stop=(k == KC - 1))
            for c in range(MC):
                nc.scalar.activation(out=kT_sb[:, c, sl], in_=ps[c][:],
                                     func=AF.Identity,
                                     bias=kb_sb[:, c:c + 1], scale=1.0)

        # ---- layernorm machinery (per 1024-wide slice) ----
        def ln_stats(src, t, pfx):
            sl = slice(t * 1024, (t + 1) * 1024)
            sq = [tmp.tile([P, 1024], bf16, name=f"{pfx}sq{c}_{t}", tag="sq")
                  for c in range(MC)]
            for c in range(MC):
                nc.scalar.activation(out=sq[c][:], in_=src[:, c, sl],
                                     func=AF.Square)
            pss = pst.tile([HPC, 1024], f32, name=f"{pfx}pss{t}", tag="pss")
            psq = pst.tile([HPC, 1024], f32, name=f"{pfx}psq{t}", tag="psq")
            for c in range(MC):
                for j in range(2):
                    jsl = slice(j * 512, (j + 1) * 512)
                    ssl = slice(t * 1024 + j * 512, t * 1024 + (j + 1) * 512)
                    nc.tensor.matmul(pss[:, jsl], sel_sb[:, c, :],
                                     src[:, c, ssl],
                                     start=(c == 0), stop=(c == MC - 1))
                    nc.tensor.matmul(psq[:, jsl], sel_sb[:, c, :],
                                     sq[c][:, jsl],
                                     start=(c == 0), stop=(c == MC - 1))
            return pss, psq

        def ln_smalls(pss, psq, t, pfx):
            musq = tmp.tile([HPC, 1024], f32, name=f"{pfx}ms{t}", tag="ms",
                            bufs=2)
            Af = tmp.tile([HPC, 1024], f32, name=f"{pfx}Af{t}", tag="Af", bufs=2)
            Ab = tmp.tile([HPC, 1024], bf16, name=f"{pfx}Ab{t}", tag="Ab", bufs=3)
            Bb = tmp.tile([HPC + 1, 1024], bf16, name=f"{pfx}Bb{t}", tag="Bb",
                          bufs=3)
            nc.scalar.activation(out=musq[:], in_=pss[:], func=AF.Square)
            nc.vector.scalar_tensor_tensor(
                out=musq[:], in0=psq[:], scalar=1.0, in1=musq[:],
                op0=OP.mult, op1=OP.subtract)          # var = E[x^2] - mu^2
            nc.scalar.activation(out=musq[:], in_=musq[:], func=AF.Ln,
                                 bias=eps4[:], scale=1.0)
            nc.scalar.activation(out=Af[:], in_=musq[:], func=AF.Exp,
                                 scale=-0.5)           # rstd = (var+eps)^-0.5
            nc.vector.tensor_copy(Ab[:], Af[:])
            # row 4 must stay 1.0 (pairs with selwb's bias row)
            nc.vector.memset(Bb[:], 1.0)
            nc.vector.scalar_tensor_tensor(
                out=Bb[0:HPC, :], in0=Af[:], scalar=-1.0, in1=pss[:],
                op0=OP.mult, op1=OP.mult)              # B = -mu*rstd
            return Ab, Bb

        def ln_bcast(t, c, selwb_sb, Ab, Bb, pfx):
            out = []
            for j in range(2):
                jsl = slice(j * 512, (j + 1) * 512)
                psA = pp.tile([P, 512], f32, name=f"{pfx}A{c}{t}{j}", tag="ab2")
                psB = pp.tile([P, 512], f32, name=f"{pfx}B{c}{t}{j}", tag="ab2")
                nc.tensor.matmul(psA[:], selwb_sb[0:HPC, c * P:(c + 1) * P],
                                 Ab[:, jsl], start=True, stop=True)
                nc.tensor.matmul(psB[:], selwb_sb[:, c * P:(c + 1) * P],
                                 Bb[:, jsl], start=True, stop=True)
                out.append((psA, psB))
            return out

        def ln_apply(src, t, c, psABs=None):
            for j in range(2):
                psA, psB = psABs[j]
                ssl = slice(t * 1024 + j * 512, t * 1024 + (j + 1) * 512)
                nc.vector.tensor_mul(src[:, c, ssl], src[:, c, ssl], psA[:])
                nc.vector.tensor_add(src[:, c, ssl], src[:, c, ssl], psB[:])

        def rope_t(c, qsw, t):
            sl = slice(t * 1024, (t + 1) * 1024)
            for blk in range(4):
                d_src = (blk ^ 1) * 32          # swap evens<->odds within head
                eng = nc.sync if blk % 2 == 0 else nc.scalar
                eng.dma_start(out=qsw[blk * 32:(blk + 1) * 32, sl],
                              in_=qT_sb[d_src:d_src + 32, c, sl])
            rt = rop.tile([P, 1024], bf16, name=f"rt{c}_{t}", tag="rt")
            nc.gpsimd.tensor_mul(rt[:], qsw[:, sl], sinf_sb[:, sl])
            nc.vector.tensor_mul(qT_sb[:, c, sl], qT_sb[:, c, sl],
                                 cosf_sb[:, sl])
            nc.vector.tensor_add(qT_sb[:, c, sl], qT_sb[:, c, sl], rt[:])

        # ---- pre-attention schedule ----
        qproj_half(0, xt0)
        kproj_n(0)
        qst0 = ln_stats(qT_sb, 0, "q")
        kproj_n(1)
        qproj_half(1, xt1)
        qst1 = ln_stats(qT_sb, 1, "q")
        kst0 = ln_stats(kT_sb, 0, "k")
        qab0 = ln_smalls(*qst0, 0, "q")
        qab1 = ln_smalls(*qst1, 1, "q")
        kab0 = ln_smalls(*kst0, 0, "k")

        qsws = [rop.tile([P, S1], bf16, name=f"qsw{c}", tag=f"qsw{c}", bufs=1)
                for c in range(MC)]

        # c0: apply from PSUM + rope, laddered for earliest attention start
        ln_apply(qT_sb, 0, 0, psABs=ln_bcast(0, 0, qselwb_sb, *qab0, "q"))
        ln_apply(kT_sb, 0, 0, psABs=ln_bcast(0, 0, kselwb_sb, *kab0, "k"))
        rope_t(0, qsws[0], 0)
        ln_apply(qT_sb, 1, 0, psABs=ln_bcast(1, 0, qselwb_sb, *qab1, "q"))
        rope_t(0, qsws[0], 1)

        # c1: broadcast now, evacuate to SBUF (so PSUM banks free before
        # attention), apply+rope later -- overlapped with attention on c0
        def bcast_evac(t, c, selwb_sb, Ab, Bb, pfx):
            asb = rop.tile([P, 1024], bf16, name=f"{pfx}as{c}{t}", tag="asb",
                           bufs=3)
            bsb = rop.tile([P, 1024], bf16, name=f"{pfx}bs{c}{t}", tag="bsb",
                           bufs=3)
            for j in range(2):
                jsl = slice(j * 512, (j + 1) * 512)
                psA = pp.tile([P, 512], f32, name=f"{pfx}A{c}{t}{j}", tag="ab2")
                psB = pp.tile([P, 512], f32, name=f"{pfx}B{c}{t}{j}", tag="ab2")
                nc.tensor.matmul(psA[:], selwb_sb[0:HPC, c * P:(c + 1) * P],
                                 Ab[:, jsl], start=True, stop=True)
                nc.tensor.matmul(psB[:], selwb_sb[:, c * P:(c + 1) * P],
                                 Bb[:, jsl], start=True, stop=True)
                nc.vector.tensor_copy(asb[:, jsl], psA[:])
                nc.vector.tensor_copy(bsb[:, jsl], psB[:])
            return asb, bsb

        c1ab = [bcast_evac(0, 1, qselwb_sb, *qab0, "q"),
                bcast_evac(1, 1, qselwb_sb, *qab1, "q"),
                bcast_evac(0, 1, kselwb_sb, *kab0, "k")]

        def apply_sbuf(src, t, c, asb, bsb):
            sl = slice(t * 1024, (t + 1) * 1024)
            nc.vector.tensor_mul(src[:, c, sl], src[:, c, sl], asb[:])
            nc.vector.tensor_add(src[:, c, sl], src[:, c, sl], bsb[:])

        # ---- v projection (bias-free; bias folds into normalize) ----
        for m in range(M2):
            psv = pp.tile([P, CPC], f32, name=f"psv{m}", tag="pp")
            for k in range(KC):
                nc.tensor.matmul(
                    psv[:], yT_sb[:, k, m * P:(m + 1) * P], vwt_sb[:, k, :],
                    start=(k == 0), stop=(k == KC - 1))
            nc.scalar.copy(out=v_sb[:, m, :, 0:HD],
                           in_=psv.rearrange("p (h d) -> p h d", h=HPC))

        # ---- attention ----
        ctxA.close()
        ctxB = ctx.enter_context(ExitStack())
        psc = ctxB.enter_context(tc.tile_pool(name="psc", bufs=1, space="PSUM"))
        pso = ctxB.enter_context(tc.tile_pool(name="pso", bufs=1, space="PSUM"))
        coll = pers.tile([P, 64], bf16)
        rcolf = pers.tile([P, 64], f32)
        rcol = pers.tile([P, 64], bf16)
        onms = [rop.tile([HD, S1], bf16, name=f"onm{c}", tag=f"onm{c}", bufs=1)
                for c in range(MC)]

        def attn_pair(c, s1h):
            base = s1h * 1024
            pso_t = [pso.tile([HD + 1, 1024], f32, name=f"pso{h2}_{c}{s1h}",
                              tag=f"pso{h2}") for h2 in range(2)]
            ets = {}
            for m in range(M2):
                psc_t = [psc.tile([P, 1024], f32, name=f"psc{h2}_{c}{s1h}{m}",
                                  tag=f"psc{h2}") for h2 in range(2)]
                for j in range(2):
                    nsl = slice(base + j * 512, base + (j + 1) * 512)
                    for h2 in range(2):
                        d0 = h2 * 64
                        nc.tensor.matmul(
                            psc_t[h2][:, j * 512:(j + 1) * 512],
                            kT_sb[d0:d0 + 64, c, m * P:(m + 1) * P],
                            qT_sb[d0:d0 + 64, c, nsl],
                            start=True, stop=True)
                for h2 in range(2):
                    et = expp.tile([P, 1024], bf16,
                                   name=f"et{h2}_{c}{s1h}{m}", tag=f"e{h2}")
                    nc.scalar.activation(out=et[:], in_=psc_t[h2][:],
                                         func=AF.Exp, scale=SCALE)
                    ets[(h2, m)] = et
                if m > 0:
                    for h2 in range(2):
                        for j in range(2):
                            nc.tensor.matmul(
                                pso_t[h2][:, j * 512:(j + 1) * 512],
                                v_sb[:, m - 1, c * 2 + h2, :],
                                ets[(h2, m - 1)][:, j * 512:(j + 1) * 512],
                                start=(m - 1 == 0), stop=False)
                    del ets[(0, m - 1)], ets[(1, m - 1)]
            for h2 in range(2):
                for j in range(2):
                    nc.tensor.matmul(
                        pso_t[h2][:, j * 512:(j + 1) * 512],
                        v_sb[:, M2 - 1, c * 2 + h2, :],
                        ets[(h2, M2 - 1)][:, j * 512:(j + 1) * 512],
                        start=False, stop=True)
            # evacuate (bf16) + denominators (two heads share one recip slot)
            rb = (c * 2 + s1h) * 32
            o_pair = []
            for h2 in range(2):
                h = c * 2 + h2
                o_sb = rop.tile([HD + 1, 1024], bf16, name=f"osb{h}_{s1h}",
                                tag=f"osb{h2}", bufs=2)
                o_pair.append(o_sb)
                nc.vector.tensor_copy(o_sb[:], pso_t[h2][:])
                r0 = rb + h2 * 16
                nc.gpsimd.dma_start(out=coll[r0:r0 + 16, :],
                                    in_=o_sb[HD:HD + 1, :])
            nc.vector.reciprocal(rcolf[rb:rb + 32, :], coll[rb:rb + 32, :])
            nc.vector.tensor_copy(rcol[rb:rb + 32, :], rcolf[rb:rb + 32, :])
            # bounce to DRAM so normalize can broadcast-read it
            nc.gpsimd.dma_start(out=rscr[rb:rb + 32, :], in_=rcol[rb:rb + 32, :])
            # normalize right away: o/den + vb per head
            sl = slice(base, base + 1024)
            for h2 in range(2):
                h = c * 2 + h2
                rbc = rop.tile([HD, 1024], bf16, name=f"rbc{h}_{s1h}",
                               tag="rbc", bufs=3)
                r0 = rb + h2 * 16
                rc_ap = rscr[r0:r0 + 16, :]
                nc.gpsimd.dma_start(
                    out=rbc,
                    in_=bass.AP(tensor=rc_ap.tensor, offset=rc_ap.offset,
                                ap=[[0, HD]] + list(rc_ap.ap)))
                dst = onorm[0:HD, c, sl] if h2 == 0 else onms[c][:, sl]
                nc.vector.tensor_mul(dst, o_pair[h2][0:HD, :], rbc[:])
                nc.vector.tensor_scalar_add(dst, dst, vbhd_sb[:, h:h + 1])
            if s1h == 1:
                nc.scalar.dma_start(out=onorm[HD:P, c, :], in_=onms[c][:])

        attn_pair(0, 0)
        # c1 applies + rope run on DVE/GPSIMD underneath attention on c0
        apply_sbuf(qT_sb, 0, 1, *c1ab[0])
        apply_sbuf(qT_sb, 1, 1, *c1ab[1])
        apply_sbuf(kT_sb, 0, 1, *c1ab[2])
        rope_t(1, qsws[1], 0)
        rope_t(1, qsws[1], 1)
        attn_pair(0, 1)
        attn_pair(1, 0)
        attn_pair(1, 1)

        # ---- output projection (partial over this core's channels) ----
        ctxB.close()
        pout = ctx.enter_context(tc.tile_pool(name="pout", bufs=4, space="PSUM"))
        for mo in range(KC):
            for t in range(2):
                ost = xs.tile([P, 1024], f16, name=f"ost{mo}_{t}", tag="ost",
                              bufs=4)
                for n in range(2):
                    sl = slice(t * 1024 + n * 512, t * 1024 + (n + 1) * 512)
                    po = pout.tile([P, 512], f32, name=f"po{mo}_{t}{n}",
                                   tag="pout")
                    for c in range(MC):
                        nc.tensor.matmul(po[:],
                                         owt_sb[:, c, mo * P:(mo + 1) * P],
                                         onorm[:, c, sl],
                                         start=(c == 0), stop=(c == MC - 1))
                    osl = slice(n * 512, (n + 1) * 512)
                    if n == 0:
                        nc.scalar.copy(out=ost[:, osl], in_=po[:])
                    else:
                        nc.vector.tensor_copy(ost[:, osl], po[:])
                eng = nc.sync if (mo * 2 + t) % 2 == 0 else nc.gpsimd
                eng.dma_start(
                    out=outT[mo * P:(mo + 1) * P,
                             t * 1024:(t + 1) * 1024], in_=ost[:])

    _legalize_waits(nc, mybir, limit=1)
    return nc


def get_nc():
    if "nc" not in _NC_CACHE:
        _NC_CACHE["nc"] = _build_nc()
    return _NC_CACHE["nc"]


def make_in_maps(x, y, q_w, q_b, kv_w, kv_b, qn_w, qn_b, kn_w, kn_b, out_w, out_b):
    import ml_dtypes
    bf = ml_dtypes.bfloat16
    perm = np.concatenate([np.arange(0, HD, 2), np.arange(1, HD, 2)])
    inv_freq = (1.0 / (10000.0 ** (np.arange(0, HD, 2, dtype=np.float32)
                                   / np.float32(HD)))).astype(np.float32)
    ang = np.arange(S1, dtype=np.float32)[None, :] * inv_freq[:, None]
    cos = np.cos(ang).astype(np.float32)           # (32, S1)
    sin = np.sin(ang).astype(np.float32)
    cosf = np.tile(cos, (4, 1))
    sinf = np.concatenate([-sin, sin, -sin, sin])
    trig = np.ascontiguousarray(
        np.stack([cosf, sinf], axis=1)).astype(bf)   # (P, 2, S1)
    sel = np.zeros((CPC, HPC), np.float32)
    for h in range(HPC):
        sel[h * HD:(h + 1) * HD, h] = 1.0 / HD
    sel = sel.astype(bf)

    def selwb(w, b):
        # rows 0-3: head-selector scaled by the (permuted) LN weight;
        # row 4: the (permuted) LN bias — paired with Bb's ones row.
        m = np.zeros((5, CPC), np.float32)
        wp, bp = w[perm], b[perm]
        for h in range(HPC):
            m[h, h * HD:(h + 1) * HD] = wp
        m[4, :] = np.tile(bp, HPC)
        return m.astype(bf)

    qselwb = selwb(qn_w.astype(np.float32), qn_b.astype(np.float32))
    kselwb = selwb(kn_w.astype(np.float32), kn_b.astype(np.float32))

    in_maps = []
    for core in range(8):
        b, g = divmod(core, 4)
        heads = [HPC * g + i for i in range(HPC)]
        qrows = np.concatenate([h * HD + perm for h in heads])
        vrows = np.concatenate([CIN + h * HD + np.arange(HD) for h in heads])
        ocols = np.concatenate([h * HD + np.arange(HD) for h in heads])
        qbp = q_b[qrows].reshape(MC, P).T            # (P, MC)
        kbp = kv_b[qrows].reshape(MC, P).T
        qkb = np.ascontiguousarray(
            np.stack([qbp, kbp], axis=1)).astype(np.float32)  # (P, 2, MC)
        vbhd = np.ascontiguousarray(
            kv_b[vrows].reshape(HPC, HD).T).astype(np.float32)  # (HD, HPC)
        in_maps.append({
            "xT": np.ascontiguousarray(x[b].T).astype(bf),
            "yT": np.ascontiguousarray(y[b].T).astype(bf),
            "qwt": np.ascontiguousarray(q_w[qrows].T).astype(bf),
            "kwt": np.ascontiguousarray(kv_w[qrows].T).astype(bf),
            "vwt": np.ascontiguousarray(kv_w[vrows].T).astype(bf),
            "owt": np.ascontiguousarray(out_w[:, ocols].T).astype(bf),
            "qkb": qkb, "vbhd": vbhd, "trig": trig, "sel": sel,
            "qselwb": qselwb, "kselwb": kselwb,
        })
    return in_maps


def assemble(parts, out_b):
    result = np.empty((B, S1, CIN), np.float32)
    for b in range(B):
        acc = parts[b * 4].astype(np.float32)
        for g in range(1, 4):
            acc = acc + parts[b * 4 + g].astype(np.float32)
        result[b] = acc.T + out_b[None, :].astype(np.float32)
    return result


def kernel(**inputs):
    args = {k: np.asarray(inputs[k], np.float32) for k in
            ("x", "y", "q_w", "q_b", "kv_w", "kv_b", "qn_w", "qn_b",
             "kn_w", "kn_b", "out_w", "out_b")}
    in_maps = make_in_maps(
        args["x"], args["y"], args["q_w"], args["q_b"], args["kv_w"],
        args["kv_b"], args["qn_w"], args["qn_b"], args["kn_w"], args["kn_b"],
        args["out_w"], args["out_b"])
    from concourse.bass_utils import run_bass_kernel_spmd
    nc = get_nc()
    res = run_bass_kernel_spmd(nc, in_maps, core_ids=list(range(8)))
    parts = [r["outT"] for r in res.results]
    return assemble(parts, args["out_b"])


# revision 25
# speedup vs baseline: 1.2694x; 1.2694x over previous
"""Trainium2 Bass kernel for nn_CrossAttention (b=2, s1=2048, s2=1024, H=16, hd=64).

Sharding: 8 cores = 2 batches x 4 head-groups (4 heads each).
Per-core device program (bf16 matmul datapath, fp32 PSUM accumulation):
  - PE warmup burst first so the HAM clock-gate opens (2.4 GHz) before the
    real matmul stream begins; qT proj weights+activations race down two
    DMA rings (sync: even cin chunks, scalar: odd).
  - qT proj: qT[c, s1] = q_w_g @ x_b^T            (channel-partition layout)
  - kT proj: kT[c, s2] = k_w_g @ y_b^T
  - v  proj: v[s2, c]  = y_b @ v_w_g^T            (seq-partition layout)
  - QK layernorm over head_dim (= partitions): stats via selector-matmul
    (1/hd prescaled on host so pss == mu, psq == E[x^2]);
    rstd = Exp(-0.5*Ln(var+eps)) on ACT -- no reciprocal, no sqrt table
    set; ln/exp/square/identity all live in one ACT table set with the
    softmax exp. Per-position affine (A = rstd, B' = w*B + b) broadcast to
    128 partitions via K=4/K=5 matmuls whose selectors carry the LN
    weight/bias (host-folded); the apply is two DVE passes from PSUM.
  - PSUM evacuations with bias fused run on ACT (Identity+bias); RoPE's
    sin-multiply runs on GPSIMD; cos-multiply + add on DVE (sign baked
    into the sin table); the even<->odd swap is an SBUF->SBUF DMA.
  - Attention per head-pair (c): psc_a/psc_b [128,1024] + pso_a/pso_b
    [65,1024] = exactly 8 PSUM banks; per s2-chunk the two heads' score
    matmuls alternate PE row-tiles (0,0)/(64,0) and the exp stream keeps
    ACT saturated; AV accumulation for chunk m-1 runs in PE slack.
  - softmax denominator = ones-column of v_aug; normalization by a
    DMA-broadcast reciprocal (bounced through DRAM for the stride-0 read).
  - out-projT partial [cout, s1] = o_wt-slices @ onorm, stored fp16;
    host sums 4 partials in fp32.

The emitted BIR is post-processed to split multi-semaphore waits into
single-wait NOP chains (this walrus build allows only 1 sync-wait on
self-loading matmults and very few elsewhere).
"""

import numpy as np

B, S1, S2, CIN, H, HD = 2, 2048, 1024, 1024, 16, 64
HPC = 4                # heads per core
CPC = HPC * HD         # 256 channels per core
P = 128
KC = CIN // P          # 8 cin chunks
MC = CPC // P          # 2 channel chunks
NQ = S1 // 512         # 4 s1 slices
NK = S2 // 512         # 2 s2 slices
M2 = S2 // P           # 8 s2 chunks
SCALE = HD ** -0.5
EPS = 1e-6

_NC_CACHE = {}


def _legalize_waits(nc, mybir, limit=1):
    """Split instructions carrying >limit semaphore waits into a chain of
    single-wait NOPs on the same engine followed by the instruction."""
    n_split = 0
    for fn in nc.m.functions:
        for bb in fn.blocks:
            out = []
            for inst in bb.instructions:
                si = inst.sync_info
                waits = list(si.on_wait) if si is not None and si.on_wait else []
                if len(waits) > limit:
                    for i, w in enumerate(waits[:-limit]):
                        nop = mybir.InstNoOp(
                            name=f"{inst.name}-lw{i}", ins=[], outs=[])
                        nop.engine = inst.engine
                        nop.sync_info = mybir.SyncInfo(on_wait=[w], on_update=[])
                        try:
                            nc.register_instruction(nop, overwrite=True)
                        except Exception:
                            pass
                        out.append(nop)
                    inst.sync_info = mybir.SyncInfo(
                        on_wait=waits[-limit:], on_update=list(si.on_update))
                    n_split += 1
                out.append(inst)
            bb.instructions = out
    return n_split


def _build_nc():
    from contextlib import ExitStack

    import concourse.bass as bass
    import concourse.mybir as mybir
    import concourse.tile as tile

    f32 = mybir.dt.float32
    f16 = mybir.dt.float16
    bf16 = mybir.dt.bfloat16
    AF = mybir.ActivationFunctionType
    OP = mybir.AluOpType

    nc = bass.Bass()

    def din(name, shape, dt=bf16):
        return nc.dram_tensor(name, list(shape), dt, kind="ExternalInput")

    xT = din("xT", (CIN, S1))
    yT = din("yT", (CIN, S2))
    qwt = din("qwt", (CIN, CPC))
    kwt = din("kwt", (CIN, CPC))
    vwt = din("vwt", (CIN, CPC))
    owt = din("owt", (CPC, CIN))
    qb = din("qb", (CPC,), f32)
    kb = din("kb", (CPC,), f32)
    vb = din("vb", (CPC,), f32)
    cosf = din("cosf", (P, S1))
    sinf = din("sinf", (P, S1))
    seld = din("sel", (CPC, HPC))           # 1/HD prescaled head selector
    qselwb = din("qselwb", (5, CPC))        # rows 0-3: w-scaled bcast sel,
    kselwb = din("kselwb", (5, CPC))        # row 4: LN bias b
    outT = nc.dram_tensor("outT", [CIN, S1], f16, kind="ExternalOutput")
    rscr = nc.dram_tensor("rscr", [P, 64], bf16, kind="Internal")

    with tile.TileContext(nc) as tc, ExitStack() as ctx:
        consts = ctx.enter_context(tc.tile_pool(name="consts", bufs=1))
        pers = ctx.enter_context(tc.tile_pool(name="pers", bufs=1))
        xs = ctx.enter_context(tc.tile_pool(name="xs", bufs=8))
        tmp = ctx.enter_context(tc.tile_pool(name="tmp", bufs=3))
        rop = ctx.enter_context(tc.tile_pool(name="rop", bufs=3))
        expp = ctx.enter_context(tc.tile_pool(name="expp", bufs=4))
        ctxA = ctx.enter_context(ExitStack())
        pp = ctxA.enter_context(tc.tile_pool(name="pp", bufs=2, space="PSUM"))
        pst = ctxA.enter_context(tc.tile_pool(name="pst", bufs=1, space="PSUM"))

        # ---- PE warmup: dummy matmuls open the HAM clock gate ----
        wdum = consts.tile([P, P], bf16)
        nc.vector.memset(wdum, 0.0)
        wps = pp.tile([P, P], f32, name="warm", tag="pp")
        for i in range(40):
            nc.tensor.matmul(wps[:], wdum[:], wdum[:], start=True, stop=True)

        # ---- constants ----
        sel_sb = consts.tile([P, MC, HPC], bf16)
        nc.scalar.dma_start(out=sel_sb, in_=seld.rearrange("(c p) h -> p c h", p=P))
        qselwb_sb = consts.tile([5, CPC], bf16)
        nc.scalar.dma_start(out=qselwb_sb, in_=qselwb[:])
        kselwb_sb = consts.tile([5, CPC], bf16)
        nc.scalar.dma_start(out=kselwb_sb, in_=kselwb[:])
        qwt_sb = consts.tile([P, KC, CPC], bf16)
        for k in range(KC):
            nc.sync.dma_start(out=qwt_sb[:, k, :], in_=qwt[k * P:(k + 1) * P, :])
        yT_sb = consts.tile([P, KC, S2], bf16)
        kwt_sb = consts.tile([P, KC, CPC], bf16)
        vwt_sb = consts.tile([P, KC, CPC], bf16)
        # gpsimd ring: rope tables, owt, per-partition consts
        cosf_sb = consts.tile([P, S1], bf16)
        nc.gpsimd.dma_start(out=cosf_sb, in_=cosf[:])
        sinf_sb = consts.tile([P, S1], bf16)
        nc.gpsimd.dma_start(out=sinf_sb, in_=sinf[:])
        owt_sb = consts.tile([P, MC, CIN], bf16)
        nc.gpsimd.dma_start(out=owt_sb, in_=owt.rearrange("(c p) m -> p c m", p=P))

        def perpart(name, d):
            t = consts.tile([P, MC], f32, name=name)
            nc.gpsimd.dma_start(out=t, in_=d.rearrange("(c p) -> p c", p=P))
            return t

        qb_sb = perpart("qb_sb", qb)
        kb_sb = perpart("kb_sb", kb)
        vbb_sb = consts.tile([P, CPC], f32)
        vb_ap = vb[:]
        nc.gpsimd.dma_start(
            out=vbb_sb,
            in_=bass.AP(tensor=vb_ap.tensor, offset=vb_ap.offset,
                        ap=[[0, P]] + list(vb_ap.ap)),
        )
        eps4 = consts.tile([HPC, 1], f32)
        nc.vector.memset(eps4, EPS)

        # ---- persistent activations ----
        qT_sb = pers.tile([P, MC, S1], bf16)
        kT_sb = pers.tile([P, MC, S2], bf16)
        v_sb = pers.tile([P, M2, HPC, HD + 1], bf16)
        onorm = pers.tile([P, MC, S1], bf16)

        # ---- q projection: qT[c*128+p, s1] ----
        # c-major so only 2 PSUM accumulators are live; the x stream is
        # split across the sync (even k) and scalar (odd k) rings and both
        # c-chunks reuse the same xt tiles.
        for half in range(2):
            hsl = slice(half * 1024, (half + 1) * 1024)
            xts = []
            for k in range(KC):
                xt = xs.tile([P, 1024], bf16, name=f"xt{half}_{k}", tag="xs")
                eng = nc.sync if k % 2 == 0 else nc.scalar
                eng.dma_start(out=xt, in_=xT[k * P:(k + 1) * P, hsl])
                xts.append(xt)
            for c in range(MC):
                ps = [pp.tile([P, 512], f32, name=f"psq{c}_{half}{n}", tag="pp")
                      for n in range(2)]
                for k in range(KC):
                    for n in range(2):
                        nc.tensor.matmul(
                            ps[n][:], qwt_sb[:, k, c * P:(c + 1) * P],
                            xts[k][:, n * 512:(n + 1) * 512],
                            start=(k == 0), stop=(k == KC - 1))
                for n in range(2):
                    sl = slice(half * 1024 + n * 512, half * 1024 + (n + 1) * 512)
                    nc.scalar.activation(out=qT_sb[:, c, sl], in_=ps[n][:],
                                         func=AF.Identity,
                                         bias=qb_sb[:, c:c + 1], scale=1.0)

        # k/v-path loads follow the x stream on the scalar ring
        for k in range(KC):
            nc.scalar.dma_start(out=yT_sb[:, k, :], in_=yT[k * P:(k + 1) * P, :])
        nc.scalar.dma_start(out=kwt_sb, in_=kwt.rearrange("(k p) m -> p k m", p=P))
        nc.scalar.dma_start(out=vwt_sb, in_=vwt.rearrange("(k p) m -> p k m", p=P))

        # ---- layernorm machinery (per 1024-wide slice) ----
        def ln_stats(src, t, pfx):
            sl = slice(t * 1024, (t + 1) * 1024)
            sq = [tmp.tile([P, 1024], bf16, name=f"{pfx}sq{c}_{t}", tag="sq")
                  for c in range(MC)]
            for c in range(MC):
                nc.scalar.activation(out=sq[c][:], in_=src[:, c, sl],
                                     func=AF.Square)
            pss = pst.tile([HPC, 1024], f32, name=f"{pfx}pss{t}", tag="pss")
            psq = pst.tile([HPC, 1024], f32, name=f"{pfx}psq{t}", tag="psq")
            for c in range(MC):
                for j in range(2):
                    jsl = slice(j * 512, (j + 1) * 512)
                    ssl = slice(t * 1024 + j * 512, t * 1024 + (j + 1) * 512)
                    nc.tensor.matmul(pss[:, jsl], sel_sb[:, c, :],
                                     src[:, c, ssl],
                                     start=(c == 0), stop=(c == MC - 1))
                    nc.tensor.matmul(psq[:, jsl], sel_sb[:, c, :],
                                     sq[c][:, jsl],
                                     start=(c == 0), stop=(c == MC - 1))
            return pss, psq

        def ln_smalls(pss, psq, t, pfx):
            musq = tmp.tile([HPC, 1024], f32, name=f"{pfx}ms{t}", tag="ms",
                            bufs=2)
            Af = tmp.tile([HPC, 1024], f32, name=f"{pfx}Af{t}", tag="Af", bufs=2)
            Ab = tmp.tile([HPC, 1024], bf16, name=f"{pfx}Ab{t}", tag="Ab", bufs=3)
            Bb = tmp.tile([HPC + 1, 1024], bf16, name=f"{pfx}Bb{t}", tag="Bb",
                          bufs=3)
            nc.scalar.activation(out=musq[:], in_=pss[:], func=AF.Square)
            nc.vector.scalar_tensor_tensor(
                out=musq[:], in0=psq[:], scalar=1.0, in1=musq[:],
                op0=OP.mult, op1=OP.subtract)          # var = E[x^2] - mu^2
            nc.scalar.activation(out=musq[:], in_=musq[:], func=AF.Ln,
                                 bias=eps4[:], scale=1.0)
            nc.scalar.activation(out=Af[:], in_=musq[:], func=AF.Exp,
                                 scale=-0.5)           # rstd = (var+eps)^-0.5
            nc.vector.tensor_copy(Ab[:], Af[:])
            # row 4 must stay 1.0 (pairs with selwb's bias row)
            nc.vector.memset(Bb[:], 1.0)
            nc.vector.scalar_tensor_tensor(
                out=Bb[0:HPC, :], in0=Af[:], scalar=-1.0, in1=pss[:],
                op0=OP.mult, op1=OP.mult)              # B = -mu*rstd
            return Ab, Bb

        def ln_bcast_apply(src, t, c, selwb_sb, Ab, Bb, pfx):
            for j in range(2):
                jsl = slice(j * 512, (j + 1) * 512)
                ssl = slice(t * 1024 + j * 512, t * 1024 + (j + 1) * 512)
                psA = pp.tile([P, 512], f32, name=f"{pfx}A{c}{t}{j}", tag="ab2")
                psB = pp.tile([P, 512], f32, name=f"{pfx}B{c}{t}{j}", tag="ab2")
                nc.tensor.matmul(psA[:], selwb_sb[0:HPC, c * P:(c + 1) * P],
                                 Ab[:, jsl], start=True, stop=True)
                nc.tensor.matmul(psB[:], selwb_sb[:, c * P:(c + 1) * P],
                                 Bb[:, jsl], start=True, stop=True)
                nc.vector.tensor_mul(src[:, c, ssl], src[:, c, ssl], psA[:])
                nc.vector.tensor_add(src[:, c, ssl], src[:, c, ssl], psB[:])

        # PE keeps rolling: stats-q-t0, kproj, stats-q-t1, vproj, stats-k;
        # ACT/DVE crunch the smalls under the projection matmuls.
        qst0 = ln_stats(qT_sb, 0, "q")

        for n in range(NK):
            sl = slice(n * 512, (n + 1) * 512)
            ps = [pp.tile([P, 512], f32, name=f"psk{c}_{n}", tag="pp")
                  for c in range(MC)]
            for k in range(KC):
                for c in range(MC):
                    nc.tensor.matmul(
                        ps[c][:], kwt_sb[:, k, c * P:(c + 1) * P],
                        yT_sb[:, k, sl],
                        start=(k == 0), stop=(k == KC - 1))
            for c in range(MC):
                nc.scalar.activation(out=kT_sb[:, c, sl], in_=ps[c][:],
                                     func=AF.Identity,
                                     bias=kb_sb[:, c:c + 1], scale=1.0)

        qst1 = ln_stats(qT_sb, 1, "q")
        qab0 = ln_smalls(*qst0, 0, "q")

        # ---- v projection: v[s2-part, channel] + ones column ----
        for m in range(M2):
            psv = pp.tile([P, CPC], f32, name=f"psv{m}", tag="pp")
            for k in range(KC):
                nc.tensor.matmul(
                    psv[:], yT_sb[:, k, m * P:(m + 1) * P], vwt_sb[:, k, :],
                    start=(k == 0), stop=(k == KC - 1))
            nc.vector.tensor_add(
                v_sb[:, m, :, 0:HD],
                psv.rearrange("p (h d) -> p h d", h=HPC),
                vbb_sb.rearrange("p (h d) -> p h d", h=HPC))
            nc.vector.memset(v_sb[:, m, :, HD:HD + 1], 1.0)

        kst0 = ln_stats(kT_sb, 0, "k")
        qab1 = ln_smalls(*qst1, 1, "q")
        kab0 = ln_smalls(*kst0, 0, "k")

        # c0 applies + RoPE first so attention's first head-pair can start
        def rope(c):
            qsw = rop.tile([P, S1], bf16, name=f"qsw{c}", tag="qsw", bufs=1)
            for blk in range(4):
                d_src = (blk ^ 1) * 32          # swap evens<->odds within head
                nc.scalar.dma_start(out=qsw[blk * 32:(blk + 1) * 32, :],
                                    in_=qT_sb[d_src:d_src + 32, c, :])
            for t in range(2):
                sl = slice(t * 1024, (t + 1) * 1024)
                rt = rop.tile([P, 1024], bf16, name=f"rt{c}_{t}", tag="rt")
                nc.gpsimd.tensor_mul(rt[:], qsw[:, sl], sinf_sb[:, sl])
                nc.vector.tensor_mul(qT_sb[:, c, sl], qT_sb[:, c, sl],
                                     cosf_sb[:, sl])
                nc.vector.tensor_add(qT_sb[:, c, sl], qT_sb[:, c, sl], rt[:])

        for c in range(MC):
            ln_bcast_apply(qT_sb, 0, c, qselwb_sb, *qab0, "q")
            ln_bcast_apply(qT_sb, 1, c, qselwb_sb, *qab1, "q")
            ln_bcast_apply(kT_sb, 0, c, kselwb_sb, *kab0, "k")
            rope(c)

        # ---- attention, one head-pair (c) and s1-half at a time ----
        # PSUM: psc_a + psc_b [128,1024] (2 banks each) + pso_a + pso_b
        # [65,1024] (2 banks each) = all 8 banks. Per s2-chunk m the two
        # heads' score matmuls alternate row-tiles (0,0)/(64,0); exp(m)
        # streams on ACT while PE runs the AV accumulation of chunk m-1.
        ctxA.close()
        ctxB = ctx.enter_context(ExitStack())
        psc = ctxB.enter_context(tc.tile_pool(name="psc", bufs=1, space="PSUM"))
        pso = ctxB.enter_context(tc.tile_pool(name="pso", bufs=1, space="PSUM"))
        coll = pers.tile([P, 64], bf16)
        rcolf = pers.tile([P, 64], f32)
        rcol = pers.tile([P, 64], bf16)
        o_sbs = {}
        for c in range(MC):
            for s1h in range(2):
                base = s1h * 1024
                pso_t = [pso.tile([HD + 1, 1024], f32, name=f"pso{h2}_{c}{s1h}",
                                  tag=f"pso{h2}") for h2 in range(2)]
                ets = {}
                for m in range(M2):
                    psc_t = [psc.tile([P, 1024], f32, name=f"psc{h2}_{c}{s1h}{m}",
                                      tag=f"psc{h2}") for h2 in range(2)]
                    for j in range(2):
                        nsl = slice(base + j * 512, base + (j + 1) * 512)
                        for h2 in range(2):
                            d0 = h2 * 64
                            nc.tensor.matmul(
                                psc_t[h2][:, j * 512:(j + 1) * 512],
                                kT_sb[d0:d0 + 64, c, m * P:(m + 1) * P],
                                qT_sb[d0:d0 + 64, c, nsl],
                                start=True, stop=True)
                    for h2 in range(2):
                        et = expp.tile([P, 1024], bf16,
                                       name=f"et{h2}_{c}{s1h}{m}", tag=f"e{h2}",
                                       bufs=2)
                        nc.scalar.activation(out=et[:], in_=psc_t[h2][:],
                                             func=AF.Exp, scale=SCALE)
                        ets[(h2, m)] = et
                    if m > 0:
                        for h2 in range(2):
                            for j in range(2):
                                nc.tensor.matmul(
                                    pso_t[h2][:, j * 512:(j + 1) * 512],
                                    v_sb[:, m - 1, c * 2 + h2, :],
                                    ets[(h2, m - 1)][:, j * 512:(j + 1) * 512],
                                    start=(m - 1 == 0), stop=False)
                        del ets[(0, m - 1)], ets[(1, m - 1)]
                for h2 in range(2):
                    for j in range(2):
                        nc.tensor.matmul(
                            pso_t[h2][:, j * 512:(j + 1) * 512],
                            v_sb[:, M2 - 1, c * 2 + h2, :],
                            ets[(h2, M2 - 1)][:, j * 512:(j + 1) * 512],
                            start=False, stop=True)
                # evacuate (bf16) + collect denominators for the pair
                # (the two heads share one 32-aligned recip slot)
                rb = (c * 2 + s1h) * 32
                for h2 in range(2):
                    h = c * 2 + h2
                    o_sb = rop.tile([HD + 1, 1024], bf16, name=f"osb{h}_{s1h}",
                                    tag=f"osb{h2}", bufs=4)
                    o_sbs[(h, s1h)] = o_sb
                    nc.vector.tensor_copy(o_sb[:], pso_t[h2][:])
                    r0 = rb + h2 * 16
                    nc.gpsimd.dma_start(out=coll[r0:r0 + 16, :],
                                        in_=o_sb[HD:HD + 1, :])
                nc.vector.reciprocal(rcolf[rb:rb + 32, :], coll[rb:rb + 32, :])
                nc.vector.tensor_copy(rcol[rb:rb + 32, :], rcolf[rb:rb + 32, :])
                # bounce to DRAM so normalize can broadcast-read it
                # (stride-0 partition APs are DRAM-source only)
                nc.gpsimd.dma_start(out=rscr[rb:rb + 32, :],
                                    in_=rcol[rb:rb + 32, :])

        # ---- normalize all heads (DMA-broadcast reciprocal, DVE multiply) ----
        for c in range(MC):
            if c == 0:
                onm0 = rop.tile([HD, S1], bf16, name="onm0", tag="onm", bufs=1)
            else:
                onm1 = rop.tile([HD, S1], bf16, name="onm1", tag="onm2", bufs=1)
            for h2 in range(2):
                h = c * 2 + h2
                for s1h in range(2):
                    sl = slice(s1h * 1024, (s1h + 1) * 1024)
                    rbc = rop.tile([HD, 1024], bf16, name=f"rbc{h}_{s1h}",
                                   tag="rbc", bufs=3)
                    r0 = (c * 2 + s1h) * 32 + h2 * 16
                    rc_ap = rscr[r0:r0 + 16, :]
                    nc.gpsimd.dma_start(
                        out=rbc,
                        in_=bass.AP(tensor=rc_ap.tensor, offset=rc_ap.offset,
                                    ap=[[0, HD]] + list(rc_ap.ap)))
                    o_sb = o_sbs[(h, s1h)]
                    if h2 == 0:
                        nc.vector.tensor_mul(onorm[0:HD, c, sl],
                                             o_sb[0:HD, :], rbc[:])
                    else:
                        onm = onm0 if c == 0 else onm1
                        nc.vector.tensor_mul(onm[:, sl], o_sb[0:HD, :], rbc[:])
            if c == 0:
                nc.scalar.dma_start(out=onorm[HD:P, 0, :], in_=onm0[:])
            else:
                nc.scalar.dma_start(out=onorm[HD:P, 1, :], in_=onm1[:])

        # ---- output projection (partial over this core's channels) ----
        ctxB.close()
        pout = ctx.enter_context(tc.tile_pool(name="pout", bufs=4, space="PSUM"))
        for mo in range(KC):
            for t in range(2):
                ost = xs.tile([P, 1024], f16, name=f"ost{mo}_{t}", tag="ost",
                              bufs=4)
                for n in range(2):
                    sl = slice(t * 1024 + n * 512, t * 1024 + (n + 1) * 512)
                    po = pout.tile([P, 512], f32, name=f"po{mo}_{t}{n}",
                                   tag="pout")
                    for c in range(MC):
                        nc.tensor.matmul(po[:],
                                         owt_sb[:, c, mo * P:(mo + 1) * P],
                                         onorm[:, c, sl],
                                         start=(c == 0), stop=(c == MC - 1))
                    osl = slice(n * 512, (n + 1) * 512)
                    if n == 0:
                        nc.scalar.copy(out=ost[:, osl], in_=po[:])
                    else:
                        nc.vector.tensor_copy(ost[:, osl], po[:])
                eng = nc.sync if (mo * 2 + t) % 2 == 0 else nc.gpsimd
                eng.dma_start(
                    out=outT[mo * P:(mo + 1) * P,
                             t * 1024:(t + 1) * 1024], in_=ost[:])

    _legalize_waits(nc, mybir, limit=1)
    return nc


def get_nc():
    if "nc" not in _NC_CACHE:
        _NC_CACHE["nc"] = _build_nc()
    return _NC_CACHE["nc"]


def make_in_maps(x, y, q_w, q_b, kv_w, kv_b, qn_w, qn_b, kn_w, kn_b, out_w, out_b):
    import ml_dtypes
    bf = ml_dtypes.bfloat16
    perm = np.concatenate([np.arange(0, HD, 2), np.arange(1, HD, 2)])
    inv_freq = (1.0 / (10000.0 ** (np.arange(0, HD, 2, dtype=np.float32)
                                   / np.float32(HD)))).astype(np.float32)
    ang = np.arange(S1, dtype=np.float32)[None, :] * inv_freq[:, None]
    cos = np.cos(ang).astype(np.float32)           # (32, S1)
    sin = np.sin(ang).astype(np.float32)
    cosf = np.tile(cos, (4, 1)).astype(bf)
    sinf = np.concatenate([-sin, sin, -sin, sin]).astype(bf)
    sel = np.zeros((CPC, HPC), np.float32)
    for h in range(HPC):
        sel[h * HD:(h + 1) * HD, h] = 1.0 / HD
    sel = sel.astype(bf)

    def selwb(w, b):
        # rows 0-3: head-selector scaled by the (permuted) LN weight;
        # row 4: the (permuted) LN bias — paired with Bb's ones row.
        m = np.zeros((5, CPC), np.float32)
        wp, bp = w[perm], b[perm]
        for h in range(HPC):
            m[h, h * HD:(h + 1) * HD] = wp
        m[4, :] = np.tile(bp, HPC)
        return m.astype(bf)

    qselwb = selwb(qn_w.astype(np.float32), qn_b.astype(np.float32))
    kselwb = selwb(kn_w.astype(np.float32), kn_b.astype(np.float32))

    in_maps = []
    for core in range(8):
        b, g = divmod(core, 4)
        heads = [HPC * g + i for i in range(HPC)]
        qrows = np.concatenate([h * HD + perm for h in heads])
        vrows = np.concatenate([CIN + h * HD + np.arange(HD) for h in heads])
        ocols = np.concatenate([h * HD + np.arange(HD) for h in heads])
        in_maps.append({
            "xT": np.ascontiguousarray(x[b].T).astype(bf),
            "yT": np.ascontiguousarray(y[b].T).astype(bf),
            "qwt": np.ascontiguousarray(q_w[qrows].T).astype(bf),
            "kwt": np.ascontiguousarray(kv_w[qrows].T).astype(bf),
            "vwt": np.ascontiguousarray(kv_w[vrows].T).astype(bf),
            "owt": np.ascontiguousarray(out_w[:, ocols].T).astype(bf),
            "qb": np.ascontiguousarray(q_b[qrows]),
            "kb": np.ascontiguousarray(kv_b[qrows]),
            "vb": np.ascontiguousarray(kv_b[vrows]),
            "cosf": cosf, "sinf": sinf, "sel": sel,
            "qselwb": qselwb, "kselwb": kselwb,
        })
    return in_maps


def assemble(parts, out_b):
    result = np.empty((B, S1, CIN), np.float32)
    for b in range(B):
        acc = parts[b * 4].astype(np.float32)
        for g in range(1, 4):
            acc = acc + parts[b * 4 + g].astype(np.float32)
        result[b] = acc.T + out_b[None, :].astype(np.float32)
    return result


def kernel(**inputs):
    args = {k: np.asarray(inputs[k], np.float32) for k in
            ("x", "y", "q_w", "q_b", "kv_w", "kv_b", "qn_w", "qn_b",
             "kn_w", "kn_b", "out_w", "out_b")}
    in_maps = make_in_maps(
        args["x"], args["y"], args["q_w"], args["q_b"], args["kv_w"],
        args["kv_b"], args["qn_w"], args["qn_b"], args["kn_w"], args["kn_b"],
        args["out_w"], args["out_b"])
    from concourse.bass_utils import run_bass_kernel_spmd
    nc = get_nc()
    res = run_bass_kernel_spmd(nc, in_maps, core_ids=list(range(8)))
    parts = [r["outT"] for r in res.results]
    return assemble(parts, args["out_b"])
